# revision 13
# baseline (speedup 1.0000x reference)
"""Multi-head causal attention (B=2, T=2048, C=1024, H=16, Dh=64) on 8 TRN2 cores.

Sharding: batch x head tensor-parallel. Core i handles batch i//4 and heads
4*(i%4) .. 4*(i%4)+3. All weights and x are pre-transposed / fp16-converted on
the host, so the device does no layout work:
  1. per 512-token chunk, load xT columns and project qT/kT/v directly in the
     feature-on-partition layout (fp16 matmuls, fp32 PSUM),
  2. causal flash attention in scoresT (keys x tokens) layout; softmax
     denominators via a ones-column folded into v'; odd heads carry the ones
     column first so their denominator lands at partition 63 and their y rows
     at 64..127 -- every normalize op stays partition-aligned and two heads
     pack one [128, 512] tile,
  3. partial output projection over the core's own 256 y-features into all
     1024 output features (+bias on group-rank-0 cores, zeros elsewhere),
  4. per-chunk ReduceScatter (sum) over the 4 cores of the batch: each core
     receives its 256 output-feature rows, already fully reduced.
Host reassembles (concat feature shards per chunk, transpose to token-major).
"""

import json

import numpy as np

import concourse.bass as bass
import concourse.mybir as mybir
from concourse.tile import TileContext
from concourse.bass_utils import run_bass_kernel_spmd
from concourse.masks import make_identity, make_upper_triangular

F32 = mybir.dt.float32
F32R = mybir.dt.float32r
F16 = mybir.dt.float16

N_CORES = 8
B = 2
T = 2048          # tokens per batch (= per core)
C = 1024          # model dim
NH_CORE = 4       # heads per core
DH = 64
FEATS = NH_CORE * DH   # 256 per-core q/k/v features
CCH = 512         # attention t-chunk
NCH = T // CCH    # 4 chunks
KTILES = T // 128  # 16 k-tiles
SCALE = 1.0 / 8.0  # 1/sqrt(DH)


def _split_waits_in_bir(bir_bytes: bytes) -> bytes:
    """Workaround: installed walrus rejects >1 sync-wait per instruction."""
    bir = json.loads(bir_bytes)
    changed = False

    def rewrite(insts):
        nonlocal changed
        out = []
        for inst in insts:
            if isinstance(inst, dict):
                for v in inst.values():
                    visit(v)
                si = inst.get("sync_info")
                engine = inst.get("engine")
                if si and engine and len(si.get("on_wait") or []) > 1:
                    waits = si["on_wait"]
                    for i, w in enumerate(waits[:-1]):
                        out.append(
                            {
                                "debug": inst.get("debug", 0),
                                "engine": engine,
                                "ins": [],
                                "name": f"{inst['name']}_ws{i}",
                                "opcode": "EventSemaphore",
                                "outs": [],
                                "sync_info": {"on_update": [], "on_wait": [w]},
                            }
                        )
                    si["on_wait"] = [waits[-1]]
                    changed = True
            out.append(inst)
        insts[:] = out

    def visit(o):
        if isinstance(o, dict):
            for k, v in o.items():
                if k == "instructions" and isinstance(v, list):
                    rewrite(v)
                else:
                    visit(v)
        elif isinstance(o, list):
            for v in o:
                visit(v)

    visit(bir)
    return json.dumps(bir).encode() if changed else bir_bytes


_PATCHED = False


def _apply_walrus_workaround():
    global _PATCHED
    if _PATCHED:
        return
    import concourse.bass_utils as bass_utils
    import concourse.bass2jax as bass2jax

    orig = bass_utils.compile_bir_kernel

    def wrapped(bir_json, tmpdir, neff_name="file.neff"):
        return orig(_split_waits_in_bir(bir_json), tmpdir, neff_name)

    bass_utils.compile_bir_kernel = wrapped
    bass2jax.compile_bir_kernel = wrapped
    _PATCHED = True


def _build_program() -> bass.Bass:
    nc = bass.Bass(num_devices=N_CORES)

    xT = nc.dram_tensor("xT", [C, T], F16, kind="ExternalInput")
    # weight tensors arrive in SBUF layout: [128, 8*256] (col block k = wT
    # rows 128k..) / [128, 2*1024] (col block m = woT rows 128m..)
    wqT = nc.dram_tensor("wqT", [128, 8 * FEATS], F16, kind="ExternalInput")
    wkT = nc.dram_tensor("wkT", [128, 8 * FEATS], F16, kind="ExternalInput")
    wvT = nc.dram_tensor("wvT", [128, 8 * FEATS], F16, kind="ExternalInput")
    woT = nc.dram_tensor("woT", [128, 2 * C], F16, kind="ExternalInput")
    bo = nc.dram_tensor("bo", [128, 8], F32, kind="ExternalInput")

    partial = [nc.dram_tensor(f"partial{c}", [C, CCH], F16) for c in range(NCH)]
    outr = [nc.dram_tensor(f"outr{c}", [FEATS, CCH], F16) for c in range(NCH)]
    outc = [
        nc.dram_tensor(f"outc{c}", [FEATS, CCH], F16, kind="ExternalOutput")
        for c in range(NCH)
    ]
    groups = [[0, 1, 2, 3], [4, 5, 6, 7]]

    with TileContext(nc) as tc:
        with (
            tc.tile_pool(name="const", bufs=1) as cpool,
            tc.tile_pool(name="wts", bufs=1) as wpool,
            tc.tile_pool(name="xload", bufs=2) as xload,
            tc.tile_pool(name="qkv", bufs=1) as qkv,
            tc.tile_pool(name="qcur", bufs=2) as qcur,
            tc.tile_pool(name="expw", bufs=6) as expw,
            tc.tile_pool(name="norm", bufs=2) as norm,
            tc.tile_pool(name="ysbp", bufs=2) as ysbp,
            tc.tile_pool(name="osb", bufs=4) as osb,
            tc.tile_pool(name="pp", bufs=2, space="PSUM") as pp,
            tc.tile_pool(name="sp", bufs=2, space="PSUM") as sp,
            tc.tile_pool(name="yp", bufs=2, space="PSUM") as yp,
        ):
            # ---- startup loads, emitted in first-consumption order: the
            # modeled DMA device is serial, so transfer order ~= emission
            # order across the two HWDGE queues
            wsb = {}
            wsb["q"] = wpool.tile([128, 8 * FEATS], F16, name="w_q")
            xs_pre = []
            xb0t = [
                xload.tile([128, 4 * CCH], F16, name=f"xb0_{h}", tag=f"xb{h}")
                for h in range(2)
            ]
            for half in range(2):
                weng = nc.sync if half == 0 else nc.scalar
                xeng = nc.scalar if half == 0 else nc.sync
                weng.dma_start(
                    out=wsb["q"][:, 1024 * half : 1024 * (half + 1)],
                    in_=wqT[:, 1024 * half : 1024 * (half + 1)],
                )
                xeng.dma_start(
                    out=xb0t[half][:].rearrange("p (k f) -> p k f", f=CCH),
                    in_=xT.ap()[512 * half : 512 * (half + 1), 0:CCH]
                    .rearrange("(k p) f -> p k f", p=128),
                )
                for k in range(4):
                    xs_pre.append(xb0t[half][:, CCH * k : CCH * (k + 1)])
            for sec, wdram in (("k", wkT), ("v", wvT)):
                wt = wpool.tile([128, 8 * FEATS], F16, name=f"w_{sec}")
                for half in range(2):
                    eng = nc.sync if half == 0 else nc.scalar
                    eng.dma_start(
                        out=wt[:, 1024 * half : 1024 * (half + 1)],
                        in_=wdram[:, 1024 * half : 1024 * (half + 1)],
                    )
                wsb[sec] = wt
            wo_sb = wpool.tile([128, 2 * C], F16, name="w_o")
            for m in range(2):
                eng = nc.sync if m == 0 else nc.scalar
                eng.dma_start(
                    out=wo_sb[:, C * m : C * (m + 1)],
                    in_=woT[:, C * m : C * (m + 1)],
                )

            # ---- constants (needed from the v'-transpose / first exp on) ----
            identity = cpool.tile([128, 128], F16)
            make_identity(nc, identity[:])
            # PE warm-up: the p-state ramp needs ~3us of continuous PE busy
            # before full clock; burn it on dummy transposes during the
            # startup DMA wait so the first real matmuls run at 2.4GHz
            warm_in = cpool.tile([128, 128], F16)
            nc.vector.memset(warm_in[:], 0.0)
            warm_ps = pp.tile([128, 128], F16, name="warmps", tag="pp")
            for _ in range(56):
                nc.tensor.matmul(
                    warm_ps[:], warm_in[:], identity[:],
                    is_transpose=True, skip_group_check=True,
                )
            mask = cpool.tile([128, 128], F16)
            make_upper_triangular(nc, mask[:], val=1.0, diag=True)
            ones_r = cpool.tile([128, 64], F32R)
            nc.vector.memset(ones_r[:].bitcast(F32), 1.0)
            bias_sb = cpool.tile([128, 8], F32)
            nc.sync.dma_start(out=bias_sb[:], in_=bo[:, :])

            # ---- persistent activations ----
            kT = [qkv.tile([128, T], F16, name=f"kT_{m}") for m in range(2)]
            # v' tiles, one per key-tile j: [128 keys, 4 heads x (64 v | 1)];
            # the ones col folds the softmax denominator into the AV matmul
            vpj = {}
            vp = {}
            for j in range(KTILES):
                t = qkv.tile([128, 4 * (DH + 1)], F16, name=f"vp_{j}")
                for h in range(NH_CORE):
                    nc.vector.memset(
                        t[:, (DH + 1) * h + DH : (DH + 1) * (h + 1)], 1.0
                    )
                    vp[h, j] = t[:, (DH + 1) * h : (DH + 1) * (h + 1)]
                vpj[j] = t

            def prefetch_x(n):
                t0 = CCH * n
                xss = []
                for half in range(2):
                    xb = xload.tile(
                        [128, 4 * CCH], F16, name=f"xb{n}_{half}", tag=f"xb{half}"
                    )
                    nc.scalar.dma_start(
                        out=xb[:].rearrange("p (k f) -> p k f", f=CCH),
                        in_=xT.ap()[512 * half : 512 * (half + 1), t0 : t0 + CCH]
                        .rearrange("(k p) f -> p k f", p=128),
                    )
                    for k in range(4):
                        xss.append(xb[:, CCH * k : CCH * (k + 1)])
                return xss

            def project_chunk(n, xss):
                """Project tokens [512n, 512n+512): qT/kT columns, v' tiles."""
                t0 = CCH * n
                qT = []
                for sec in ("q", "k"):
                    for m in range(2):
                        ps = pp.tile([128, CCH], F32, name="projps", tag="pp")
                        for k in range(8):
                            nc.tensor.matmul(
                                ps[:],
                                wsb[sec][:, 256 * k + 128 * m : 256 * k + 128 * (m + 1)],
                                xss[k][:],
                                start=(k == 0),
                                stop=(k == 7),
                            )
                        if sec == "q":
                            qt = qkv.tile([128, CCH], F16, name=f"qT{n}_{m}")
                            nc.scalar.copy(out=qt[:], in_=ps[:])
                            qT.append(qt)
                        else:
                            nc.vector.tensor_copy(
                                out=kT[m][:, t0 : t0 + CCH], in_=ps[:]
                            )
                # v directly in key-major layout: stationary = xT k-tile
                # (tokens as PE columns), moving = wv -> out[token, feat].
                # No transpose anywhere; one strided DVE copy per key-tile
                # scatters the 4 heads' 64-col blocks into the v' layout.
                for tb in range(4):
                    j = 4 * n + tb
                    ps = pp.tile([128, 4 * DH], F32, name="vtps", tag="pp")
                    for k in range(8):
                        nc.tensor.matmul(
                            ps[:],
                            xss[k][:, 128 * tb : 128 * (tb + 1)],
                            wsb["v"][:, 256 * k : 256 * (k + 1)],
                            start=(k == 0),
                            stop=(k == 7),
                        )
                    nc.vector.tensor_copy(
                        out=vpj[j][:]
                        .rearrange("p (h e) -> p h e", e=DH + 1)[:, :, 0:DH],
                        in_=ps[:].rearrange("p (h d) -> p h d", d=DH),
                    )
                return qT

            def attend_chunk(c, qT):
                """Attention for tokens [512c, 512c+512), all heads + ysb."""
                jlast = 4 * c + 3
                ysb = [
                    ysbp.tile([128, CCH], F16, name=f"ysb{c}_{m}", tag=f"ysb{m}")
                    for m in range(2)
                ]
                # Heads run in interleaved PAIRS: while one head's exp is in
                # flight the PE streams the other head's matmuls, hiding the
                # score->exp->AV latency. Odd head leads (its ysb rows need a
                # partition-shifting SBUF->SBUF DMA, which then overlaps).
                def head_ctx(h):
                    m, b_ = h // 2, h % 2
                    ytp_t = yp.tile([128, CCH], F32, name=f"ytp{c}_{h}", tag="ytp")
                    return {
                        "h": h, "m": m, "b": b_,
                        "hq": qT[m][64 * b_ : 64 * (b_ + 1), :],
                        "hk": kT[m][64 * b_ : 64 * (b_ + 1), :],
                        "ytp_t": ytp_t,
                    }

                def attend_pair(ctx, p):
                    # two j-tiles share one PSUM score tile and one exp
                    # instruction: halves the Act engine's per-instruction
                    # access-latency overhead, which paces the pipeline
                    sc = sp.tile([128, 1024], F32, name="sc", tag="sc")
                    ex = expw.tile([128, 1024], F16, name="ex", tag="ex")
                    info = []
                    off = 0
                    for half in range(2):
                        j = 2 * p + half
                        tstart = max(128 * j, CCH * c)
                        w = CCH * (c + 1) - tstart
                        nc.tensor.matmul(
                            sc[0:128, off : off + w],
                            ctx["hk"][:, 128 * j : 128 * (j + 1)],
                            ctx["hq"][:, tstart - CCH * c : tstart - CCH * c + w],
                            start=True,
                            stop=True,
                        )
                        info.append((j, tstart, w, off))
                        off += w
                    nc.scalar.activation(
                        ex[:, 0:off],
                        sc[0:128, 0:off],
                        mybir.ActivationFunctionType.Exp,
                        scale=SCALE,
                    )
                    for j, tstart, w, o in info:
                        if 128 * j >= CCH * c:
                            nc.vector.tensor_mul(
                                out=ex[:, o : o + 128],
                                in0=ex[:, o : o + 128],
                                in1=mask[:],
                            )
                        lo = tstart - CCH * c
                        h = ctx["h"]
                        nc.tensor.matmul(
                            ctx["ytp_t"][0 : DH + 1, lo : lo + w],
                            vpj[j][:, (DH + 1) * h : (DH + 1) * (h + 1)],
                            ex[:, o : o + w],
                            start=(j == 0),
                            stop=(j == jlast),
                        )

                def normalize(ctx):
                    # bc rides the sc pool's rotation (same slot size, no
                    # extra PSUM banks) -- keeping it out of the pp pool lets
                    # next-chunk projection matmuls fill attention gaps
                    m, b_, ytp_t = ctx["m"], ctx["b"], ctx["ytp_t"]
                    den = norm.tile([128, CCH], F32R, name="den", tag="den")
                    nc.vector.tensor_copy(out=den[64:65, :], in_=ytp_t[64:65, :])
                    bc = sp.tile([64, CCH], F32, name="bc", tag="sc")
                    nc.tensor.matmul(
                        bc[:], ones_r[64:65, :], den[64:65, :],
                        start=True, stop=True,
                    )
                    bcr = norm.tile([64, CCH], F32, name="bcr", tag="bcr")
                    nc.vector.reciprocal(bcr[:], bc[:])
                    if b_ == 0:
                        nc.vector.tensor_mul(
                            out=ysb[m][0:64, :], in0=ytp_t[0:DH, :], in1=bcr[:]
                        )
                    else:
                        ysh = norm.tile([64, CCH], F16, name="ysh", tag="ysh")
                        nc.vector.tensor_mul(
                            out=ysh[:], in0=ytp_t[0:DH, :], in1=bcr[:]
                        )
                        nc.gpsimd.dma_start(out=ysb[m][64:128, :], in_=ysh[:])

                for h in (1, 0, 3, 2):
                    ctx = head_ctx(h)
                    for p in range(2 * c + 2):
                        attend_pair(ctx, p)
                    normalize(ctx)
                return ysb

            def out_proj(c, ysb):
                """Partial out-projection (own 256 y-feats -> all 1024 outs).

                PSUM->SBUF(+bias) copies split across DVE (t=0) and the
                Activation engine (t=1, Identity+bias) so the post-attention
                epilogue runs on two engines in parallel; each half DMAs from
                its own HWDGE queue.
                """
                for t in range(4):
                    # 2 m-tiles per flush: one epilogue copy on DVE, one on
                    # the Act engine (parallel), then a quarter-DMA -- the
                    # last flush after the final matmul is short
                    ob = osb.tile([128, 2 * CCH], F16, name=f"ob{c}_{t}", tag="ob")
                    for s in range(2):
                        o = 2 * t + s
                        ps = pp.tile([128, CCH], F32, name="ops", tag="pp")
                        for m in range(2):
                            nc.tensor.matmul(
                                ps[:],
                                wo_sb[:, C * m + 128 * o : C * m + 128 * (o + 1)],
                                ysb[m][:],
                                start=(m == 0),
                                stop=(m == 1),
                            )
                        if s == 0:
                            nc.vector.tensor_scalar_add(
                                out=ob[:, CCH * s : CCH * (s + 1)],
                                in0=ps[:],
                                scalar1=bias_sb[:, o : o + 1],
                            )
                        else:
                            nc.scalar.activation(
                                ob[:, CCH * s : CCH * (s + 1)],
                                ps[:],
                                mybir.ActivationFunctionType.Identity,
                                bias=bias_sb[:, o : o + 1],
                            )
                    eng = nc.sync if t % 2 == 0 else nc.scalar
                    eng.dma_start(
                        out=partial[c]
                        .ap()[256 * t : 256 * (t + 1), :]
                        .rearrange("(k p) f -> p k f", p=128),
                        in_=ob[:].rearrange("p (k f) -> p k f", f=CCH),
                    )

            def reduce_scatter(c):
                # walrus forbids collectives writing IO tensors -> bounce via
                # outr, then copy (both HWDGE queues for the last chunk)
                nc.gpsimd.collective_compute(
                    "ReduceScatter",
                    mybir.AluOpType.add,
                    replica_groups=groups,
                    ins=[partial[c][:].opt()],
                    outs=[outr[c][:].opt()],
                )
                if c < NCH - 1:
                    nc.sync.dma_start(out=outc[c][:, :], in_=outr[c][:, :])
                else:
                    nc.sync.dma_start(
                        out=outc[c][0:128, :], in_=outr[c][0:128, :]
                    )
                    nc.scalar.dma_start(
                        out=outc[c][128:256, :], in_=outr[c][128:256, :]
                    )

            # proj(c+1) is emitted BEFORE out_proj(c): its PSUM tiles then
            # rotate ahead of pout's in the pp pool, so the scheduler can
            # slot next-chunk projection matmuls into the PE gaps of the
            # Act-engine-paced attention stretch
            qT = project_chunk(0, xs_pre)
            for c in range(NCH):
                if c + 1 < NCH:
                    xss_next = prefetch_x(c + 1)
                ysb = attend_chunk(c, qT)
                if c + 1 < NCH:
                    qT = project_chunk(c + 1, xss_next)
                out_proj(c, ysb)
                reduce_scatter(c)

    return nc


_PROGRAM = None


def _get_program():
    global _PROGRAM
    if _PROGRAM is None:
        _apply_walrus_workaround()
        _PROGRAM = _build_program()
    return _PROGRAM


def kernel(x, w_qkv, w_out, b_out):
    x = np.asarray(x, dtype=np.float32)
    w_qkv = np.asarray(w_qkv, dtype=np.float32)
    w_out = np.asarray(w_out, dtype=np.float32)
    b_out = np.asarray(b_out, dtype=np.float32)

    bias_tile = np.ascontiguousarray(b_out.reshape(8, 128).T)  # [128, 8]
    zeros_tile = np.zeros_like(bias_tile)

    def sb_layout(wT):  # [1024or256, F] -> [128, (k f)] SBUF layout
        k = wT.shape[0] // 128
        return np.ascontiguousarray(
            wT.reshape(k, 128, -1).transpose(1, 0, 2).reshape(128, -1)
        )

    in_maps = []
    for i in range(N_CORES):
        b, g = divmod(i, 4)
        sl = slice(FEATS * g, FEATS * (g + 1))
        in_maps.append(
            {
                "xT": np.ascontiguousarray(x[b].T.astype(np.float16)),
                "wqT": sb_layout(w_qkv[0 * C :][sl].T.astype(np.float16)),
                "wkT": sb_layout(w_qkv[1 * C :][sl].T.astype(np.float16)),
                "wvT": sb_layout(w_qkv[2 * C :][sl].T.astype(np.float16)),
                "woT": sb_layout(w_out[:, sl].T.astype(np.float16)),
                "bo": bias_tile if g == 0 else zeros_tile,
            }
        )

    nc = _get_program()
    res = run_bass_kernel_spmd(nc, in_maps, core_ids=list(range(N_CORES)))
    kernel.last_results = res

    outs = []
    for b in range(B):
        full = np.empty((C, T), dtype=np.float32)
        for g in range(4):
            r = res.results[4 * b + g]
            for c in range(NCH):
                full[FEATS * g : FEATS * (g + 1), CCH * c : CCH * (c + 1)] = r[
                    f"outc{c}"
                ].astype(np.float32)
        outs.append(full.T)
    return np.stack(outs)



# revision 25
# speedup vs baseline: 1.1301x; 1.1301x over previous
"""Multi-head causal attention (B=2, T=2048, C=1024, H=16, Dh=64) on 8 TRN2 cores.

Sharding: batch x head tensor-parallel. Core i handles batch i//4 and heads
4*(i%4) .. 4*(i%4)+3. All weights and x are pre-transposed / fp16-converted on
the host, so the device does no layout work:
  1. per 512-token chunk, load xT columns and project qT/kT/v directly in the
     feature-on-partition layout (fp16 matmuls, fp32 PSUM),
  2. causal flash attention in scoresT (keys x tokens) layout; softmax
     denominators via a ones-column folded into v'; odd heads carry the ones
     column first so their denominator lands at partition 63 and their y rows
     at 64..127 -- every normalize op stays partition-aligned and two heads
     pack one [128, 512] tile,
  3. partial output projection over the core's own 256 y-features into all
     1024 output features (+bias on group-rank-0 cores, zeros elsewhere),
  4. per-chunk ReduceScatter (sum) over the 4 cores of the batch: each core
     receives its 256 output-feature rows, already fully reduced.
Host reassembles (concat feature shards per chunk, transpose to token-major).
"""

import json

import numpy as np

import concourse.bass as bass
import concourse.mybir as mybir
from concourse.tile import TileContext
from concourse.bass_utils import run_bass_kernel_spmd
from concourse.masks import make_identity, make_upper_triangular

F32 = mybir.dt.float32
F32R = mybir.dt.float32r
F16 = mybir.dt.float16

N_CORES = 8
B = 2
T = 2048          # tokens per batch (= per core)
C = 1024          # model dim
NH_CORE = 4       # heads per core
DH = 64
FEATS = NH_CORE * DH   # 256 per-core q/k/v features
CCH = 512         # attention t-chunk
NCH = T // CCH    # 4 chunks
KTILES = T // 128  # 16 k-tiles
SCALE = 1.0 / 8.0  # 1/sqrt(DH)


def _split_waits_in_bir(bir_bytes: bytes) -> bytes:
    """Workaround: installed walrus rejects >1 sync-wait per instruction."""
    bir = json.loads(bir_bytes)
    changed = False

    def rewrite(insts):
        nonlocal changed
        out = []
        for inst in insts:
            if isinstance(inst, dict):
                for v in inst.values():
                    visit(v)
                si = inst.get("sync_info")
                engine = inst.get("engine")
                if si and engine and len(si.get("on_wait") or []) > 1:
                    waits = si["on_wait"]
                    for i, w in enumerate(waits[:-1]):
                        out.append(
                            {
                                "debug": inst.get("debug", 0),
                                "engine": engine,
                                "ins": [],
                                "name": f"{inst['name']}_ws{i}",
                                "opcode": "EventSemaphore",
                                "outs": [],
                                "sync_info": {"on_update": [], "on_wait": [w]},
                            }
                        )
                    si["on_wait"] = [waits[-1]]
                    changed = True
            out.append(inst)
        insts[:] = out

    def visit(o):
        if isinstance(o, dict):
            for k, v in o.items():
                if k == "instructions" and isinstance(v, list):
                    rewrite(v)
                else:
                    visit(v)
        elif isinstance(o, list):
            for v in o:
                visit(v)

    visit(bir)
    return json.dumps(bir).encode() if changed else bir_bytes


_PATCHED = False


def _apply_walrus_workaround():
    global _PATCHED
    if _PATCHED:
        return
    import concourse.bass_utils as bass_utils
    import concourse.bass2jax as bass2jax

    orig = bass_utils.compile_bir_kernel

    def wrapped(bir_json, tmpdir, neff_name="file.neff"):
        return orig(_split_waits_in_bir(bir_json), tmpdir, neff_name)

    bass_utils.compile_bir_kernel = wrapped
    bass2jax.compile_bir_kernel = wrapped
    _PATCHED = True


def _build_program() -> bass.Bass:
    nc = bass.Bass(num_devices=N_CORES)

    xT = nc.dram_tensor("xT", [C, T], F16, kind="ExternalInput")
    # weight tensors arrive in SBUF layout: [128, 8*256] (col block k = wT
    # rows 128k..) / [128, 2*1024] (col block m = woT rows 128m..)
    wqT = nc.dram_tensor("wqT", [128, 8 * FEATS], F16, kind="ExternalInput")
    wkT = nc.dram_tensor("wkT", [128, 8 * FEATS], F16, kind="ExternalInput")
    wvT = nc.dram_tensor("wvT", [128, 8 * FEATS], F16, kind="ExternalInput")
    woT = nc.dram_tensor("woT", [128, 2 * C], F16, kind="ExternalInput")
    bo = nc.dram_tensor("bo", [128, 8], F32, kind="ExternalInput")

    partial = [nc.dram_tensor(f"partial{c}", [C, CCH], F16) for c in range(NCH)]
    outr = [nc.dram_tensor(f"outr{c}", [FEATS, CCH], F16) for c in range(NCH)]
    outc = [
        nc.dram_tensor(f"outc{c}", [FEATS, CCH], F16, kind="ExternalOutput")
        for c in range(NCH)
    ]
    groups = [[0, 1, 2, 3], [4, 5, 6, 7]]

    with TileContext(nc) as tc:
        with (
            tc.tile_pool(name="const", bufs=1) as cpool,
            tc.tile_pool(name="wts", bufs=1) as wpool,
            tc.tile_pool(name="xload", bufs=3) as xload,
            tc.tile_pool(name="qkv", bufs=1) as qkv,
            tc.tile_pool(name="qcur", bufs=2) as qcur,
            tc.tile_pool(name="expw", bufs=2) as expw,
            tc.tile_pool(name="norm", bufs=2) as norm,
            tc.tile_pool(name="ysbp", bufs=2) as ysbp,
            tc.tile_pool(name="osb", bufs=4) as osb,
            tc.tile_pool(name="pp", bufs=2, space="PSUM") as pp,
            tc.tile_pool(name="sp", bufs=2, space="PSUM") as sp,
            tc.tile_pool(name="yp", bufs=2, space="PSUM") as yp,
        ):
            # ---- startup loads, emitted in first-consumption order: the
            # modeled DMA device is serial, so transfer order ~= emission
            # order across the two HWDGE queues
            wsb = {}
            wsb["q"] = wpool.tile([128, 8 * FEATS], F16, name="w_q")
            xs_pre = []
            xb0t = [
                xload.tile([128, 4 * CCH], F16, name=f"xb0_{h}", tag=f"xb{h}")
                for h in range(2)
            ]
            for half in range(2):
                weng = nc.sync if half == 0 else nc.scalar
                xeng = nc.scalar if half == 0 else nc.sync
                weng.dma_start(
                    out=wsb["q"][:, 1024 * half : 1024 * (half + 1)],
                    in_=wqT[:, 1024 * half : 1024 * (half + 1)],
                )
                xeng.dma_start(
                    out=xb0t[half][:].rearrange("p (k f) -> p k f", f=CCH),
                    in_=xT.ap()[512 * half : 512 * (half + 1), 0:CCH]
                    .rearrange("(k p) f -> p k f", p=128),
                )
                for k in range(4):
                    xs_pre.append(xb0t[half][:, CCH * k : CCH * (k + 1)])
            for sec, wdram in (("k", wkT), ("v", wvT)):
                wt = wpool.tile([128, 8 * FEATS], F16, name=f"w_{sec}")
                for half in range(2):
                    eng = nc.sync if half == 0 else nc.scalar
                    eng.dma_start(
                        out=wt[:, 1024 * half : 1024 * (half + 1)],
                        in_=wdram[:, 1024 * half : 1024 * (half + 1)],
                    )
                wsb[sec] = wt
            wo_sb = wpool.tile([128, 2 * C], F16, name="w_o")
            for m in range(2):
                eng = nc.sync if m == 0 else nc.scalar
                eng.dma_start(
                    out=wo_sb[:, C * m : C * (m + 1)],
                    in_=woT[:, C * m : C * (m + 1)],
                )

            # ---- constants (needed from the v'-transpose / first exp on) ----
            identity = cpool.tile([128, 128], F16)
            make_identity(nc, identity[:])
            # PE warm-up: the p-state ramp needs ~3us of continuous PE busy
            # before full clock; burn it on dummy transposes during the
            # startup DMA wait so the first real matmuls run at 2.4GHz
            warm_in = cpool.tile([128, 128], F16)
            nc.vector.memset(warm_in[:], 0.0)
            warm_ps = pp.tile([128, 128], F16, name="warmps", tag="pp")
            for _ in range(56):
                nc.tensor.matmul(
                    warm_ps[:], warm_in[:], identity[:],
                    is_transpose=True, skip_group_check=True,
                )
            mask = cpool.tile([128, 128], F16)
            make_upper_triangular(nc, mask[:], val=1.0, diag=True)
            bias_sb = cpool.tile([128, 8], F32)
            nc.sync.dma_start(out=bias_sb[:], in_=bo[:, :])

            # ---- persistent activations ----
            kT = [qkv.tile([128, T], F16, name=f"kT_{m}") for m in range(2)]
            # v' tiles, one per key-tile j: [128 keys, 4 heads x (64 v | 1)];
            # the ones col folds the softmax denominator into the AV matmul
            vpj = {}
            vp = {}
            for j in range(KTILES):
                t = qkv.tile([128, 4 * (DH + 1)], F16, name=f"vp_{j}")
                for h in range(NH_CORE):
                    nc.vector.memset(
                        t[:, (DH + 1) * h + DH : (DH + 1) * (h + 1)], 1.0
                    )
                    vp[h, j] = t[:, (DH + 1) * h : (DH + 1) * (h + 1)]
                vpj[j] = t

            def prefetch_x(n):
                t0 = CCH * n
                xss = []
                for half in range(2):
                    xb = xload.tile(
                        [128, 4 * CCH], F16, name=f"xb{n}_{half}", tag=f"xb{half}"
                    )
                    nc.scalar.dma_start(
                        out=xb[:].rearrange("p (k f) -> p k f", f=CCH),
                        in_=xT.ap()[512 * half : 512 * (half + 1), t0 : t0 + CCH]
                        .rearrange("(k p) f -> p k f", p=128),
                    )
                    for k in range(4):
                        xss.append(xb[:, CCH * k : CCH * (k + 1)])
                return xss

            def make_proj_fillers(n, xss):
                """qT tiles + 8 filler closures (one PSUM group each) that
                project chunk n.  Fillers are interleaved between attention
                pairs so the PE stream never drains during exp-bound spans."""
                t0 = CCH * n
                qtiles = [
                    qkv.tile([128, CCH], F16, name=f"qT{n}_{m}") for m in range(2)
                ]
                fillers = []

                def qk_group(sec, m):
                    def f():
                        ps = pp.tile([128, CCH], F32, name="projps", tag="pp")
                        for k in range(8):
                            nc.tensor.matmul(
                                ps[:],
                                wsb[sec][:, 256 * k + 128 * m : 256 * k + 128 * (m + 1)],
                                xss[k][:],
                                start=(k == 0),
                                stop=(k == 7),
                            )
                        if sec == "q":
                            nc.scalar.copy(out=qtiles[m][:], in_=ps[:])
                        else:
                            nc.vector.tensor_copy(
                                out=kT[m][:, t0 : t0 + CCH], in_=ps[:]
                            )
                    return f

                def v_group(tb):
                    # v directly in key-major layout: stationary = xT k-tile
                    # (tokens as PE columns), moving = wv -> out[token, feat];
                    # one strided DVE copy scatters the 4 heads into v'.
                    def f():
                        j = 4 * n + tb
                        ps = pp.tile([128, 4 * DH], F32, name="vtps", tag="pp")
                        for k in range(8):
                            nc.tensor.matmul(
                                ps[:],
                                xss[k][:, 128 * tb : 128 * (tb + 1)],
                                wsb["v"][:, 256 * k : 256 * (k + 1)],
                                start=(k == 0),
                                stop=(k == 7),
                            )
                        nc.vector.tensor_copy(
                            out=vpj[j][:]
                            .rearrange("p (h e) -> p h e", e=DH + 1)[:, :, 0:DH],
                            in_=ps[:].rearrange("p (h d) -> p h d", d=DH),
                        )
                    return f

                for sec in ("q", "k"):
                    for m in range(2):
                        fillers.append(qk_group(sec, m))
                for tb in range(4):
                    fillers.append(v_group(tb))
                return qtiles, fillers

            def attend_chunk(c, qT, fillers):
                """Attention for tokens [512c, 512c+512), all heads + ysb.

                Stage A (scores+exp+mask) streams per (pair, head) into
                double-buffered per-(h,p) ex tiles.  Stage B runs the AV
                matmuls TRANSPOSED (stationary = 128-token slice of ex,
                moving = [v|1]): the output lands token-major
                [128 tok, 4*(64+1)] so the modeled PE cost is 65 cols per
                key-tile instead of up-to-512, causal (j,tb) blocks above
                the diagonal are skipped entirely, and the softmax
                denominator is a per-PARTITION scalar (cheap reciprocal +
                tensor_scalar mul, no broadcast matmul).

                `fillers` (next-chunk projection groups, previous-chunk
                out-projection flushes) are interleaved between pairs so the
                in-order PE stream always has matmuls while ACT runs exp.
                Stage B token-blocks are emitted as soon as their key-range
                is complete (tb 0,1 after pair 2c; tb 2,3 after the last
                pair) instead of at the end.
                """
                ysb = [
                    ysbp.tile([128, CCH], F16, name=f"ysb{c}_{m}", tag=f"ysb{m}")
                    for m in range(2)
                ]
                exinfo = {}
                fq = list(fillers)
                npairs = 2 * c + 2

                def stage_b(tb):
                    jmax = 4 * c + tb
                    yT = yp.tile([128, 4 * (DH + 1)], F32, name=f"yT{tb}", tag="ytp")
                    for h in range(NH_CORE):
                        for j in range(jmax + 1):
                            ex, col0 = exinfo[h, j]
                            nc.tensor.matmul(
                                yT[:, (DH + 1) * h : (DH + 1) * (h + 1)],
                                ex[:, col0 + 128 * tb : col0 + 128 * (tb + 1)],
                                vpj[j][:, (DH + 1) * h : (DH + 1) * (h + 1)],
                                start=(j == 0),
                                stop=(j == jmax),
                            )
                    rec = norm.tile([128, 4], F32, name="rec", tag="rec")
                    nc.vector.reciprocal(
                        rec[:],
                        yT[:].rearrange("p (h e) -> p h e", e=DH + 1)[:, :, DH : DH + 1],
                    )
                    yn = norm.tile([128, 4 * DH], F16, name="yn", tag="yn")
                    for h in range(NH_CORE):
                        nc.vector.tensor_scalar_mul(
                            out=yn[:, DH * h : DH * (h + 1)],
                            in0=yT[:, (DH + 1) * h : (DH + 1) * h + DH],
                            scalar1=rec[:, h : h + 1],
                        )
                    # transpose back to feature-major ysb via PE (rides the
                    # pp pool rotation) + DVE copy
                    for m in range(2):
                        tp = pp.tile([128, 128], F16, name="ytr", tag="pp")
                        nc.tensor.matmul(
                            tp[:],
                            yn[:, 128 * m : 128 * (m + 1)],
                            identity[:],
                            is_transpose=True,
                        )
                        nc.vector.tensor_copy(
                            out=ysb[m][:, 128 * tb : 128 * (tb + 1)], in_=tp[:]
                        )

                for p in range(npairs):
                    for h in range(NH_CORE):
                        m, b_ = h // 2, h % 2
                        hq = qT[m][64 * b_ : 64 * (b_ + 1), :]
                        hk = kT[m][64 * b_ : 64 * (b_ + 1), :]
                        sc = sp.tile([128, 1024], F32, name="sc", tag="sc")
                        ex = expw.tile(
                            [128, 1024], F16, name=f"ex{h}_{p}", tag=f"ex{h}_{p}"
                        )
                        info = []
                        off = 0
                        for half in range(2):
                            j = 2 * p + half
                            tstart = max(128 * j, CCH * c)
                            w = CCH * (c + 1) - tstart
                            nc.tensor.matmul(
                                sc[0:128, off : off + w],
                                hk[:, 128 * j : 128 * (j + 1)],
                                hq[:, tstart - CCH * c : tstart - CCH * c + w],
                                start=True,
                                stop=True,
                            )
                            info.append((j, tstart, w, off))
                            off += w
                        nc.scalar.activation(
                            ex[:, 0:off],
                            sc[0:128, 0:off],
                            mybir.ActivationFunctionType.Exp,
                            scale=SCALE,
                        )
                        for j, tstart, w, o in info:
                            if 128 * j >= CCH * c:
                                nc.vector.tensor_mul(
                                    out=ex[:, o : o + 128],
                                    in0=ex[:, o : o + 128],
                                    in1=mask[:],
                                )
                            # col of chunk-relative token t is col0 + t
                            exinfo[h, j] = (ex, o - (tstart - CCH * c))
                    # spread fillers evenly over the remaining pairs
                    nfill = len(fq) // (npairs - p) + (1 if len(fq) % (npairs - p) else 0)
                    for _ in range(min(nfill, len(fq))):
                        fq.pop(0)()
                    if p == 2 * c:
                        stage_b(0)
                        stage_b(1)
                stage_b(2)
                stage_b(3)
                return ysb

            def make_outproj_fillers(c, ysb):
                """4 filler closures, one flush each: 2 o-groups of the
                partial out-projection (own 256 y-feats -> all 1024 outs),
                bias epilogue on DVE, then the partial DMA on the sync queue
                (the scalar queue is reserved for wait-free prefetches so exp
                dispatch on the shared Activation SEQ never stalls)."""
                def flush(t):
                    def f():
                        ob = osb.tile(
                            [128, 2 * CCH], F16, name=f"ob{c}_{t}", tag="ob"
                        )
                        for s in range(2):
                            o = 2 * t + s
                            ps = pp.tile([128, CCH], F32, name="ops", tag="pp")
                            for m in range(2):
                                nc.tensor.matmul(
                                    ps[:],
                                    wo_sb[:, C * m + 128 * o : C * m + 128 * (o + 1)],
                                    ysb[m][:],
                                    start=(m == 0),
                                    stop=(m == 1),
                                )
                            nc.vector.tensor_scalar_add(
                                out=ob[:, CCH * s : CCH * (s + 1)],
                                in0=ps[:],
                                scalar1=bias_sb[:, o : o + 1],
                            )
                        nc.sync.dma_start(
                            out=partial[c]
                            .ap()[256 * t : 256 * (t + 1), :]
                            .rearrange("(k p) f -> p k f", p=128),
                            in_=ob[:].rearrange("p (k f) -> p k f", f=CCH),
                        )
                    return f

                return [flush(t) for t in range(4)]

            def reduce_scatter(c):
                # walrus forbids collectives writing IO tensors -> bounce via
                # outr, then copy (both HWDGE queues for the last chunk)
                nc.gpsimd.collective_compute(
                    "ReduceScatter",
                    mybir.AluOpType.add,
                    replica_groups=groups,
                    ins=[partial[c][:].opt()],
                    outs=[outr[c][:].opt()],
                )
                if c < NCH - 1:
                    # Pool queue: sits right behind the RS it waits on, so it
                    # never blocks prefetch/partial dispatch on the HWDGE SEQs
                    nc.gpsimd.dma_start(out=outc[c][:, :], in_=outr[c][:, :])
                else:
                    nc.sync.dma_start(
                        out=outc[c][0:128, :], in_=outr[c][0:128, :]
                    )
                    nc.scalar.dma_start(
                        out=outc[c][128:256, :], in_=outr[c][128:256, :]
                    )

            # chunk 0's projection runs up front; thereafter each chunk's
            # attention interleaves (a) the next chunk's projection groups
            # and (b) the PREVIOUS chunk's out-projection flushes + RS as
            # PE fillers between exp-bound attention pairs
            qT, fillers0 = make_proj_fillers(0, xs_pre)
            for f in fillers0:
                f()
            prev_out = []  # outproj fillers + RS emitter for chunk c-1
            for c in range(NCH):
                fillers = list(prev_out)
                if c + 1 < NCH:
                    xss_next = prefetch_x(c + 1)
                    qT_next, pf = make_proj_fillers(c + 1, xss_next)
                    fillers += pf
                ysb = attend_chunk(c, qT, fillers)
                if c + 1 < NCH:
                    qT = qT_next
                prev_out = make_outproj_fillers(c, ysb)
                prev_out.append(lambda c=c: reduce_scatter(c))
            for f in prev_out:
                f()

    return nc


_PROGRAM = None


def _get_program():
    global _PROGRAM
    if _PROGRAM is None:
        _apply_walrus_workaround()
        _PROGRAM = _build_program()
    return _PROGRAM


def kernel(x, w_qkv, w_out, b_out):
    x = np.asarray(x, dtype=np.float32)
    w_qkv = np.asarray(w_qkv, dtype=np.float32)
    w_out = np.asarray(w_out, dtype=np.float32)
    b_out = np.asarray(b_out, dtype=np.float32)

    bias_tile = np.ascontiguousarray(b_out.reshape(8, 128).T)  # [128, 8]
    zeros_tile = np.zeros_like(bias_tile)

    def sb_layout(wT):  # [1024or256, F] -> [128, (k f)] SBUF layout
        k = wT.shape[0] // 128
        return np.ascontiguousarray(
            wT.reshape(k, 128, -1).transpose(1, 0, 2).reshape(128, -1)
        )

    in_maps = []
    for i in range(N_CORES):
        b, g = divmod(i, 4)
        sl = slice(FEATS * g, FEATS * (g + 1))
        in_maps.append(
            {
                "xT": np.ascontiguousarray(x[b].T.astype(np.float16)),
                "wqT": sb_layout(w_qkv[0 * C :][sl].T.astype(np.float16)),
                "wkT": sb_layout(w_qkv[1 * C :][sl].T.astype(np.float16)),
                "wvT": sb_layout(w_qkv[2 * C :][sl].T.astype(np.float16)),
                "woT": sb_layout(w_out[:, sl].T.astype(np.float16)),
                "bo": bias_tile if g == 0 else zeros_tile,
            }
        )

    nc = _get_program()
    res = run_bass_kernel_spmd(nc, in_maps, core_ids=list(range(N_CORES)))
    kernel.last_results = res

    outs = []
    for b in range(B):
        full = np.empty((C, T), dtype=np.float32)
        for g in range(4):
            r = res.results[4 * b + g]
            for c in range(NCH):
                full[FEATS * g : FEATS * (g + 1), CCH * c : CCH * (c + 1)] = r[
                    f"outc{c}"
                ].astype(np.float32)
        outs.append(full.T)
    return np.stack(outs)



# revision 83
# speedup vs baseline: 1.1499x; 1.0175x over previous
"""Multi-head causal attention (B=2, T=2048, C=1024, H=16, Dh=64) on 8 TRN2 cores.

Sharding: batch x head tensor-parallel. Core i handles batch i//4 and heads
4*(i%4) .. 4*(i%4)+3. All weights and x are pre-transposed / fp16-converted on
the host, so the device does no layout work:
  1. per 512-token chunk, load xT columns and project qT/kT/v directly in the
     feature-on-partition layout (fp16 matmuls, fp32 PSUM),
  2. causal flash attention in scoresT (keys x tokens) layout; softmax
     denominators via a ones-column folded into v'; odd heads carry the ones
     column first so their denominator lands at partition 63 and their y rows
     at 64..127 -- every normalize op stays partition-aligned and two heads
     pack one [128, 512] tile,
  3. partial output projection over the core's own 256 y-features into all
     1024 output features (+bias on group-rank-0 cores, zeros elsewhere),
  4. per-chunk ReduceScatter (sum) over the 4 cores of the batch: each core
     receives its 256 output-feature rows, already fully reduced.
Host reassembles (concat feature shards per chunk, transpose to token-major).
"""

import json

import numpy as np

import concourse.bass as bass
import concourse.mybir as mybir
from concourse.tile import TileContext
from concourse.bass_utils import run_bass_kernel_spmd
from concourse.masks import make_identity, make_upper_triangular

F32 = mybir.dt.float32
F32R = mybir.dt.float32r
F16 = mybir.dt.float16

N_CORES = 8
B = 2
T = 2048          # tokens per batch (= per core)
C = 1024          # model dim
NH_CORE = 4       # heads per core
DH = 64
FEATS = NH_CORE * DH   # 256 per-core q/k/v features
CCH = 512         # attention t-chunk
NCH = T // CCH    # 4 chunks
KTILES = T // 128  # 16 k-tiles
SCALE = 1.0 / 8.0  # 1/sqrt(DH)


def _split_waits_in_bir(bir_bytes: bytes) -> bytes:
    """Workaround: installed walrus rejects >1 sync-wait per instruction."""
    bir = json.loads(bir_bytes)
    changed = False

    def rewrite(insts):
        nonlocal changed
        out = []
        for inst in insts:
            if isinstance(inst, dict):
                for v in inst.values():
                    visit(v)
                si = inst.get("sync_info")
                engine = inst.get("engine")
                if si and engine and len(si.get("on_wait") or []) > 1:
                    waits = si["on_wait"]
                    for i, w in enumerate(waits[:-1]):
                        out.append(
                            {
                                "debug": inst.get("debug", 0),
                                "engine": engine,
                                "ins": [],
                                "name": f"{inst['name']}_ws{i}",
                                "opcode": "EventSemaphore",
                                "outs": [],
                                "sync_info": {"on_update": [], "on_wait": [w]},
                            }
                        )
                    si["on_wait"] = [waits[-1]]
                    changed = True
            out.append(inst)
        insts[:] = out

    def visit(o):
        if isinstance(o, dict):
            for k, v in o.items():
                if k == "instructions" and isinstance(v, list):
                    rewrite(v)
                else:
                    visit(v)
        elif isinstance(o, list):
            for v in o:
                visit(v)

    visit(bir)
    return json.dumps(bir).encode() if changed else bir_bytes


_PATCHED = False


def _apply_walrus_workaround():
    global _PATCHED
    if _PATCHED:
        return
    import concourse.bass_utils as bass_utils
    import concourse.bass2jax as bass2jax

    orig = bass_utils.compile_bir_kernel

    def wrapped(bir_json, tmpdir, neff_name="file.neff"):
        return orig(_split_waits_in_bir(bir_json), tmpdir, neff_name)

    bass_utils.compile_bir_kernel = wrapped
    bass2jax.compile_bir_kernel = wrapped
    _PATCHED = True


def _build_program() -> bass.Bass:
    nc = bass.Bass(num_devices=N_CORES)

    xT = nc.dram_tensor("xT", [C, T], F16, kind="ExternalInput")
    # weight tensors arrive in SBUF layout: [128, 8*256] (col block k = wT
    # rows 128k..) / [128, 2*1024] (col block m = woT rows 128m..)
    wqT = nc.dram_tensor("wqT", [128, 8 * FEATS], F16, kind="ExternalInput")
    wkT = nc.dram_tensor("wkT", [128, 8 * FEATS], F16, kind="ExternalInput")
    wvT = nc.dram_tensor("wvT", [128, 8 * FEATS], F16, kind="ExternalInput")
    woT = nc.dram_tensor("woT", [128, 2 * C], F16, kind="ExternalInput")
    bo = nc.dram_tensor("bo", [128, 8], F32, kind="ExternalInput")

    partial = [nc.dram_tensor(f"partial{c}", [C, CCH], F16) for c in range(NCH)]
    outr = [nc.dram_tensor(f"outr{c}", [FEATS, CCH], F16) for c in range(NCH)]
    outc = [
        nc.dram_tensor(f"outc{c}", [FEATS, CCH], F16, kind="ExternalOutput")
        for c in range(NCH)
    ]
    groups = [[0, 1, 2, 3], [4, 5, 6, 7]]

    with TileContext(nc) as tc:
        with (
            tc.tile_pool(name="const", bufs=1) as cpool,
            tc.tile_pool(name="wts", bufs=1) as wpool,
            tc.tile_pool(name="xload", bufs=3) as xload,
            tc.tile_pool(name="qkv", bufs=1) as qkv,
            tc.tile_pool(name="qcur", bufs=2) as qcur,
            tc.tile_pool(name="expw", bufs=2) as expw,
            tc.tile_pool(name="norm", bufs=2) as norm,
            tc.tile_pool(name="ysbp", bufs=3) as ysbp,
            tc.tile_pool(name="osb", bufs=3) as osb,
            tc.tile_pool(name="pp", bufs=2, space="PSUM") as pp,
            tc.tile_pool(name="sp", bufs=2, space="PSUM") as sp,
            tc.tile_pool(name="yp", bufs=2, space="PSUM") as yp,
        ):
            # ---- startup loads, emitted in first-consumption order: the
            # modeled DMA device is serial, so transfer order ~= emission
            # order across the two HWDGE queues
            wsb = {}
            wsb["q"] = wpool.tile([128, 8 * FEATS], F16, name="w_q")
            xs_pre = []
            xb0t = [
                xload.tile([128, 4 * CCH], F16, name=f"xb0_{h}", tag=f"xb{h}")
                for h in range(2)
            ]
            for half in range(2):
                weng = nc.sync if half == 0 else nc.scalar
                xeng = nc.scalar if half == 0 else nc.sync
                weng.dma_start(
                    out=wsb["q"][:, 1024 * half : 1024 * (half + 1)],
                    in_=wqT[:, 1024 * half : 1024 * (half + 1)],
                )
                xeng.dma_start(
                    out=xb0t[half][:].rearrange("p (k f) -> p k f", f=CCH),
                    in_=xT.ap()[512 * half : 512 * (half + 1), 0:CCH]
                    .rearrange("(k p) f -> p k f", p=128),
                )
                for k in range(4):
                    xs_pre.append(xb0t[half][:, CCH * k : CCH * (k + 1)])
            for sec, wdram in (("k", wkT), ("v", wvT)):
                wt = wpool.tile([128, 8 * FEATS], F16, name=f"w_{sec}")
                for half in range(2):
                    eng = nc.sync if half == 0 else nc.scalar
                    eng.dma_start(
                        out=wt[:, 1024 * half : 1024 * (half + 1)],
                        in_=wdram[:, 1024 * half : 1024 * (half + 1)],
                    )
                wsb[sec] = wt
            wo_sb = wpool.tile([128, 2 * C], F16, name="w_o")
            for m in range(2):
                eng = nc.sync if m == 0 else nc.scalar
                eng.dma_start(
                    out=wo_sb[:, C * m : C * (m + 1)],
                    in_=woT[:, C * m : C * (m + 1)],
                )

            # ---- constants (needed from the v'-transpose / first exp on) ----
            identity = cpool.tile([128, 128], F16)
            make_identity(nc, identity[:])
            # PE warm-up: the p-state ramp needs ~3us of continuous PE busy
            # before full clock; burn it on dummy transposes during the
            # startup DMA wait so the first real matmuls run at 2.4GHz
            warm_in = cpool.tile([128, 128], F16)
            nc.vector.memset(warm_in[:], 0.0)
            warm_ps = pp.tile([128, 128], F16, name="warmps", tag="pp")
            for _ in range(32):
                nc.tensor.matmul(
                    warm_ps[:], warm_in[:], identity[:],
                    is_transpose=True, skip_group_check=True,
                )
            mask = cpool.tile([128, 128], F16)
            make_upper_triangular(nc, mask[:], val=1.0, diag=True)
            bias_sb = cpool.tile([128, 8], F32)
            nc.sync.dma_start(out=bias_sb[:], in_=bo[:, :])

            # ---- persistent activations ----
            kT = [qkv.tile([128, T], F16, name=f"kT_{m}") for m in range(2)]
            # v' tiles, one per key-tile j: [128 keys, 4 heads x (64 v | 1)];
            # the ones col folds the softmax denominator into the AV matmul
            vpj = {}
            vp = {}
            for j in range(KTILES):
                t = qkv.tile([128, 4 * (DH + 1)], F16, name=f"vp_{j}")
                for h in range(NH_CORE):
                    nc.vector.memset(
                        t[:, (DH + 1) * h + DH : (DH + 1) * (h + 1)], 1.0
                    )
                    vp[h, j] = t[:, (DH + 1) * h : (DH + 1) * (h + 1)]
                vpj[j] = t

            def prefetch_x(n):
                t0 = CCH * n
                xss = []
                for half in range(2):
                    xb = xload.tile(
                        [128, 4 * CCH], F16, name=f"xb{n}_{half}", tag=f"xb{half}"
                    )
                    nc.scalar.dma_start(
                        out=xb[:].rearrange("p (k f) -> p k f", f=CCH),
                        in_=xT.ap()[512 * half : 512 * (half + 1), t0 : t0 + CCH]
                        .rearrange("(k p) f -> p k f", p=128),
                    )
                    for k in range(4):
                        xss.append(xb[:, CCH * k : CCH * (k + 1)])
                return xss

            def make_proj_fillers(n, xss):
                """qT tiles + 8 filler closures (one PSUM group each) that
                project chunk n.  Fillers are interleaved between attention
                pairs so the PE stream never drains during exp-bound spans."""
                t0 = CCH * n
                qtiles = [
                    qkv.tile([128, CCH], F16, name=f"qT{n}_{m}") for m in range(2)
                ]
                fillers = []

                def qk_group(sec, m):
                    def f():
                        ps = pp.tile([128, CCH], F32, name="projps", tag="pp")
                        for k in range(8):
                            nc.tensor.matmul(
                                ps[:],
                                wsb[sec][:, 256 * k + 128 * m : 256 * k + 128 * (m + 1)],
                                xss[k][:],
                                start=(k == 0),
                                stop=(k == 7),
                            )
                        if sec == "q":
                            nc.scalar.copy(out=qtiles[m][:], in_=ps[:])
                        else:
                            nc.vector.tensor_copy(
                                out=kT[m][:, t0 : t0 + CCH], in_=ps[:]
                            )
                    return f

                def v_group(tb):
                    # v directly in key-major layout: stationary = xT k-tile
                    # (tokens as PE columns), moving = wv -> out[token, feat];
                    # one strided DVE copy scatters the 4 heads into v'.
                    def f():
                        j = 4 * n + tb
                        ps = pp.tile([128, 4 * DH], F32, name="vtps", tag="pp")
                        for k in range(8):
                            nc.tensor.matmul(
                                ps[:],
                                xss[k][:, 128 * tb : 128 * (tb + 1)],
                                wsb["v"][:, 256 * k : 256 * (k + 1)],
                                start=(k == 0),
                                stop=(k == 7),
                            )
                        nc.vector.tensor_copy(
                            out=vpj[j][:]
                            .rearrange("p (h e) -> p h e", e=DH + 1)[:, :, 0:DH],
                            in_=ps[:].rearrange("p (h d) -> p h d", d=DH),
                        )
                    return f

                for sec in ("q", "k"):
                    for m in range(2):
                        fillers.append(qk_group(sec, m))
                for tb in range(4):
                    fillers.append(v_group(tb))
                return qtiles, fillers

            # Attention emitters.  Stage A (scores+exp+mask) streams per
            # (pair, head) into double-buffered per-(h,p) ex tiles.  Stage B
            # runs the AV matmuls TRANSPOSED (stationary = 128-token slice
            # of ex, moving = [v|1]): the output lands token-major
            # [128 tok, 4*(64+1)] so the modeled PE cost is 65 cols per
            # key-tile instead of up-to-512, causal (j,tb) blocks above the
            # diagonal are skipped entirely, and the softmax denominator is
            # a per-PARTITION scalar (cheap reciprocal + tensor_scalar mul,
            # no broadcast matmul).
            def make_attn_state(c, qT):
                return {
                    "c": c,
                    "qT": qT,
                    "exinfo": {},
                    "yT": {},
                    "ysb": [
                        ysbp.tile([128, CCH], F16, name=f"ysb{c}_{m}", tag=f"ysb{m}")
                        for m in range(2)
                    ],
                }

            def emit_pair(st, p):
                c, qT, exinfo = st["c"], st["qT"], st["exinfo"]
                for h in range(NH_CORE):
                    m, b_ = h // 2, h % 2
                    hq = qT[m][64 * b_ : 64 * (b_ + 1), :]
                    hk = kT[m][64 * b_ : 64 * (b_ + 1), :]
                    sc = sp.tile([128, 1024], F32, name="sc", tag="sc")
                    ex = expw.tile(
                        [128, 1024], F16, name=f"ex{h}_{p}", tag=f"ex{h}_{p}"
                    )
                    info = []
                    off = 0
                    for half in range(2):
                        j = 2 * p + half
                        tstart = max(128 * j, CCH * c)
                        w = CCH * (c + 1) - tstart
                        nc.tensor.matmul(
                            sc[0:128, off : off + w],
                            hk[:, 128 * j : 128 * (j + 1)],
                            hq[:, tstart - CCH * c : tstart - CCH * c + w],
                            start=True,
                            stop=True,
                        )
                        info.append((j, tstart, w, off))
                        off += w
                    nc.scalar.activation(
                        ex[:, 0:off],
                        sc[0:128, 0:off],
                        mybir.ActivationFunctionType.Exp,
                        scale=SCALE,
                    )
                    for j, tstart, w, o in info:
                        if 128 * j >= CCH * c:
                            nc.vector.tensor_mul(
                                out=ex[:, o : o + 128],
                                in0=ex[:, o : o + 128],
                                in1=mask[:],
                            )
                        # col of chunk-relative token t is col0 + t
                        exinfo[h, j] = (ex, o - (tstart - CCH * c))

            def stage_b_acc(st, tb, j_lo, j_hi):
                # accumulate key-tiles [j_lo, j_hi] into yT[tb]
                c, exinfo = st["c"], st["exinfo"]
                jmax = 4 * c + tb
                if tb not in st["yT"]:
                    st["yT"][tb] = yp.tile(
                        [128, 4 * (DH + 1)], F32, name=f"yT{tb}", tag="ytp"
                    )
                yT = st["yT"][tb]
                for h in range(NH_CORE):
                    for j in range(j_lo, j_hi + 1):
                        ex, col0 = exinfo[h, j]
                        nc.tensor.matmul(
                            yT[:, (DH + 1) * h : (DH + 1) * (h + 1)],
                            ex[:, col0 + 128 * tb : col0 + 128 * (tb + 1)],
                            vpj[j][:, (DH + 1) * h : (DH + 1) * (h + 1)],
                            start=(j == 0),
                            stop=(j == jmax),
                        )

            def stage_b_fin(st, tb):
                yT = st["yT"].pop(tb)
                ysb = st["ysb"]
                rec = norm.tile([128, 4], F32, name="rec", tag="rec")
                nc.vector.reciprocal(
                    rec[:],
                    yT[:].rearrange("p (h e) -> p h e", e=DH + 1)[:, :, DH : DH + 1],
                )
                yn = norm.tile([128, 4 * DH], F16, name="yn", tag="yn")
                for h in range(NH_CORE):
                    nc.vector.tensor_scalar_mul(
                        out=yn[:, DH * h : DH * (h + 1)],
                        in0=yT[:, (DH + 1) * h : (DH + 1) * h + DH],
                        scalar1=rec[:, h : h + 1],
                    )
                # transpose back to feature-major ysb via PE (rides the
                # pp pool rotation) + DVE copy
                for m in range(2):
                    tp = pp.tile([128, 128], F16, name="ytr", tag="pp")
                    nc.tensor.matmul(
                        tp[:],
                        yn[:, 128 * m : 128 * (m + 1)],
                        identity[:],
                        is_transpose=True,
                    )
                    nc.vector.tensor_copy(
                        out=ysb[m][:, 128 * tb : 128 * (tb + 1)], in_=tp[:]
                    )

            def stage_b(st, tb):
                stage_b_acc(st, tb, 0, 4 * st["c"] + tb)
                stage_b_fin(st, tb)

            def attend_run(st, fillers, start_pair=0, next_st=None, next_pairs=2):
                """Emit chunk st's pairs [start_pair..], interleaving fillers,
                with stage-B tb 0,1 at the causal threshold.  The NEXT
                chunk's first two pairs are emitted at the end so ACT has
                exps to run across the chunk boundary; tb 2,3 are deferred
                into the next chunk's filler stream."""
                c = st["c"]
                npairs = 2 * c + 2
                fq = list(fillers)
                for p in range(start_pair, npairs):
                    emit_pair(st, p)
                    # spread fillers evenly over the remaining pairs
                    nfill = len(fq) // (npairs - p) + (1 if len(fq) % (npairs - p) else 0)
                    for _ in range(min(nfill, len(fq))):
                        fq.pop(0)()
                    if p == 2 * c:
                        stage_b(st, 0)
                        stage_b(st, 1)
                if next_st is not None:
                    for np_ in range(next_pairs):
                        emit_pair(next_st, np_)
                return [lambda: stage_b(st, 2), lambda: stage_b(st, 3)]

            def make_outproj_fillers(c, ysb, act_epilogue=False):
                """4 filler closures, one flush each: 2 o-groups of the
                partial out-projection (own 256 y-feats -> all 1024 outs),
                bias epilogue on DVE, then the partial DMA on the sync queue
                (the scalar queue is reserved for wait-free prefetches so exp
                dispatch on the shared Activation SEQ never stalls)."""
                def flush(t):
                    def f():
                        ob = osb.tile(
                            [128, 2 * CCH], F16, name=f"ob{c}_{t}", tag="ob",
                            bufs=4,
                        )
                        for s in range(2):
                            o = 2 * t + s
                            ps = pp.tile([128, CCH], F32, name="ops", tag="pp")
                            for m in range(2):
                                nc.tensor.matmul(
                                    ps[:],
                                    wo_sb[:, C * m + 128 * o : C * m + 128 * (o + 1)],
                                    ysb[m][:],
                                    start=(m == 0),
                                    stop=(m == 1),
                                )
                            if act_epilogue and s == 1:
                                # final chunk: ACT is idle post-exp, run half
                                # the epilogue there in parallel with DVE
                                nc.scalar.activation(
                                    ob[:, CCH * s : CCH * (s + 1)],
                                    ps[:],
                                    mybir.ActivationFunctionType.Identity,
                                    bias=bias_sb[:, o : o + 1],
                                )
                            else:
                                nc.vector.tensor_scalar_add(
                                    out=ob[:, CCH * s : CCH * (s + 1)],
                                    in0=ps[:],
                                    scalar1=bias_sb[:, o : o + 1],
                                )
                        nc.sync.dma_start(
                            out=partial[c]
                            .ap()[256 * t : 256 * (t + 1), :]
                            .rearrange("(k p) f -> p k f", p=128),
                            in_=ob[:].rearrange("p (k f) -> p k f", f=CCH),
                        )
                    return f

                return [flush(t) for t in range(4)]

            def make_outproj_half(c, ysb, half):
                """Half-token out-projection flushes (tokens 256*half..):
                the first half only needs stage-B tb 0,1 so it runs mid-
                attention; the second half is the post-exp tail, with its
                epilogue split across DVE and the then-idle ACT engine."""
                def flush(t):
                    def f():
                        ob = osb.tile(
                            [128, CCH], F16, name=f"obh{c}_{t}_{half}",
                            tag=f"obh{half}", bufs=2,
                        )
                        for s in range(2):
                            o = 2 * t + s
                            # post-exp flushes borrow the score pool's PSUM
                            # slots (scores are finished) for 2x pipelining
                            pool, ptag = (sp, "sc") if half == 1 and s == 1 else (pp, "pp")
                            ps = pool.tile([128, 256], F32, name="opsh", tag=ptag)
                            for m in range(2):
                                nc.tensor.matmul(
                                    ps[:],
                                    wo_sb[:, C * m + 128 * o : C * m + 128 * (o + 1)],
                                    ysb[m][:, 256 * half : 256 * (half + 1)],
                                    start=(m == 0),
                                    stop=(m == 1),
                                )
                            if half == 1 and s == 1:
                                nc.scalar.activation(
                                    ob[:, 256 * s : 256 * (s + 1)],
                                    ps[:],
                                    mybir.ActivationFunctionType.Identity,
                                    bias=bias_sb[:, o : o + 1],
                                )
                            else:
                                nc.vector.tensor_scalar_add(
                                    out=ob[:, 256 * s : 256 * (s + 1)],
                                    in0=ps[:],
                                    scalar1=bias_sb[:, o : o + 1],
                                )
                        nc.sync.dma_start(
                            out=partial[c]
                            .ap()[256 * t : 256 * (t + 1), 256 * half : 256 * (half + 1)]
                            .rearrange("(k p) f -> p k f", p=128),
                            in_=ob[:].rearrange("p (k f) -> p k f", f=256),
                        )
                    return f

                return [flush(t) for t in range(4)]

            def reduce_scatter(c, last=False):
                # walrus forbids collectives writing IO tensors -> bounce via
                # outr, then copy (both HWDGE queues for the last chunk)
                nc.gpsimd.collective_compute(
                    "ReduceScatter",
                    mybir.AluOpType.add,
                    replica_groups=groups,
                    ins=[partial[c][:].opt()],
                    outs=[outr[c][:].opt()],
                )
                if not last:
                    # Pool queue: sits right behind the RS it waits on, so it
                    # never blocks prefetch/partial dispatch on the HWDGE SEQs
                    nc.gpsimd.dma_start(out=outc[c][:, :], in_=outr[c][:, :])
                else:
                    nc.sync.dma_start(out=outc[c][:, :], in_=outr[c][:, :])

            # Chunks are processed 1,2,3,0: the ACT-bound big chunks sit
            # mid-stream (surrounded by projection/out-projection fillers),
            # and the cheapest chunk (0) forms the tail feeding the final
            # ReduceScatter.  Each attend interleaves as PE fillers: the
            # previous chunk's deferred B-blocks + out-projection flushes +
            # its RS, and the next chunk's projection groups.
            xss = {0: xs_pre}
            qtiles = {}
            qtiles[0], pf0 = make_proj_fillers(0, xss[0])
            for f in pf0:
                f()
            sts = {0: make_attn_state(0, qtiles[0])}
            pending = []
            for c in range(NCH):
                nxt = c + 1 if c + 1 < NCH else None
                if nxt is not None:
                    xss[nxt] = prefetch_x(nxt)
                    qtiles[nxt], pf = make_proj_fillers(nxt, xss[nxt])
                    pending += pf
                    sts[nxt] = make_attn_state(nxt, qtiles[nxt])
                lookahead = {0: 2, 1: 2, 2: 4}  # pairs of chunk c+1 emitted in chunk c
                bdef = attend_run(
                    sts[c],
                    pending,
                    start_pair=0 if c == 0 else lookahead[c - 1],
                    next_st=sts.get(nxt) if nxt is not None else None,
                    next_pairs=lookahead.get(c, 0),
                )
                pending = bdef + make_outproj_fillers(
                    c, sts[c]["ysb"], act_epilogue=(nxt is None)
                )
                pending.append(
                    lambda c=c, last=(nxt is None): reduce_scatter(c, last)
                )
            for f in pending:
                f()

    return nc


_PROGRAM = None


def _get_program():
    global _PROGRAM
    if _PROGRAM is None:
        _apply_walrus_workaround()
        _PROGRAM = _build_program()
    return _PROGRAM


def kernel(x, w_qkv, w_out, b_out):
    x = np.asarray(x, dtype=np.float32)
    w_qkv = np.asarray(w_qkv, dtype=np.float32)
    w_out = np.asarray(w_out, dtype=np.float32)
    b_out = np.asarray(b_out, dtype=np.float32)

    bias_tile = np.ascontiguousarray(b_out.reshape(8, 128).T)  # [128, 8]
    zeros_tile = np.zeros_like(bias_tile)

    def sb_layout(wT):  # [1024or256, F] -> [128, (k f)] SBUF layout
        k = wT.shape[0] // 128
        return np.ascontiguousarray(
            wT.reshape(k, 128, -1).transpose(1, 0, 2).reshape(128, -1)
        )

    in_maps = []
    for i in range(N_CORES):
        b, g = divmod(i, 4)
        sl = slice(FEATS * g, FEATS * (g + 1))
        in_maps.append(
            {
                "xT": np.ascontiguousarray(x[b].T.astype(np.float16)),
                "wqT": sb_layout(w_qkv[0 * C :][sl].T.astype(np.float16)),
                "wkT": sb_layout(w_qkv[1 * C :][sl].T.astype(np.float16)),
                "wvT": sb_layout(w_qkv[2 * C :][sl].T.astype(np.float16)),
                "woT": sb_layout(w_out[:, sl].T.astype(np.float16)),
                "bo": bias_tile if g == 0 else zeros_tile,
            }
        )

    nc = _get_program()
    res = run_bass_kernel_spmd(nc, in_maps, core_ids=list(range(N_CORES)))
    kernel.last_results = res

    outs = []
    for b in range(B):
        full = np.empty((C, T), dtype=np.float32)
        for g in range(4):
            r = res.results[4 * b + g]
            for c in range(NCH):
                full[FEATS * g : FEATS * (g + 1), CCH * c : CCH * (c + 1)] = r[
                    f"outc{c}"
                ].astype(np.float32)
        outs.append(full.T)
    return np.stack(outs)



# revision 122
# speedup vs baseline: 1.1632x; 1.0116x over previous
"""Multi-head causal attention (B=2, T=2048, C=1024, H=16, Dh=64) on 8 TRN2 cores.

Sharding: batch x head tensor-parallel. Core i handles batch i//4 and heads
4*(i%4) .. 4*(i%4)+3. All weights and x are pre-transposed / fp16-converted on
the host, so the device does no layout work:
  1. per 512-token chunk, project qT/kT feature-major and v' TOKEN-major
     (stationary = xT tile), so the AV value tiles need no transpose,
  2. causal flash attention: scores in [keys x tokens] layout, exp on ACT,
     then TRANSPOSED AV (stationary = 128-token slice of the exp weights,
     moving = [v | ones]) accumulating token-major y with the softmax
     denominator as a per-partition scalar; above-diagonal (key,token)
     blocks are skipped outright,
  3. partial output projection over the core's own 256 y-features into all
     1024 output features (+bias on group-rank-0 cores, zeros elsewhere),
  4. per-chunk ReduceScatter (sum) over the 4 cores of the batch: each core
     receives its 256 output-feature rows, already fully reduced.
The emission schedule interleaves next-chunk projection groups, previous-
chunk out-projection flushes, and a 3-4 pair cross-chunk attention lookahead
between exp-bound attention pairs so the in-order PE stream never drains.
Host reassembles (concat feature shards per chunk, transpose to token-major).
"""

import json

import numpy as np

import concourse.bass as bass
import concourse.mybir as mybir
from concourse.tile import TileContext
from concourse.bass_utils import run_bass_kernel_spmd
from concourse.masks import make_identity, make_upper_triangular

F32 = mybir.dt.float32
F32R = mybir.dt.float32r
F16 = mybir.dt.float16

N_CORES = 8
B = 2
T = 2048          # tokens per batch (= per core)
C = 1024          # model dim
NH_CORE = 4       # heads per core
DH = 64
FEATS = NH_CORE * DH   # 256 per-core q/k/v features
CCH = 512         # attention t-chunk
NCH = T // CCH    # 4 chunks
KTILES = T // 128  # 16 k-tiles
SCALE = 1.0 / 8.0  # 1/sqrt(DH)


def _split_waits_in_bir(bir_bytes: bytes) -> bytes:
    """Workaround: installed walrus rejects >1 sync-wait per instruction."""
    bir = json.loads(bir_bytes)
    changed = False

    def rewrite(insts):
        nonlocal changed
        out = []
        for inst in insts:
            if isinstance(inst, dict):
                for v in inst.values():
                    visit(v)
                si = inst.get("sync_info")
                engine = inst.get("engine")
                if si and engine and len(si.get("on_wait") or []) > 1:
                    waits = si["on_wait"]
                    for i, w in enumerate(waits[:-1]):
                        out.append(
                            {
                                "debug": inst.get("debug", 0),
                                "engine": engine,
                                "ins": [],
                                "name": f"{inst['name']}_ws{i}",
                                "opcode": "EventSemaphore",
                                "outs": [],
                                "sync_info": {"on_update": [], "on_wait": [w]},
                            }
                        )
                    si["on_wait"] = [waits[-1]]
                    changed = True
            out.append(inst)
        insts[:] = out

    def visit(o):
        if isinstance(o, dict):
            for k, v in o.items():
                if k == "instructions" and isinstance(v, list):
                    rewrite(v)
                else:
                    visit(v)
        elif isinstance(o, list):
            for v in o:
                visit(v)

    visit(bir)
    return json.dumps(bir).encode() if changed else bir_bytes


_PATCHED = False


def _apply_walrus_workaround():
    global _PATCHED
    if _PATCHED:
        return
    import concourse.bass_utils as bass_utils
    import concourse.bass2jax as bass2jax

    orig = bass_utils.compile_bir_kernel

    def wrapped(bir_json, tmpdir, neff_name="file.neff"):
        return orig(_split_waits_in_bir(bir_json), tmpdir, neff_name)

    bass_utils.compile_bir_kernel = wrapped
    bass2jax.compile_bir_kernel = wrapped
    _PATCHED = True


def _build_program() -> bass.Bass:
    nc = bass.Bass(num_devices=N_CORES)

    xT = nc.dram_tensor("xT", [C, T], F16, kind="ExternalInput")
    # weight tensors arrive in SBUF layout: [128, 8*256] (col block k = wT
    # rows 128k..) / [128, 2*1024] (col block m = woT rows 128m..)
    wqT = nc.dram_tensor("wqT", [128, 8 * FEATS], F16, kind="ExternalInput")
    wkT = nc.dram_tensor("wkT", [128, 8 * FEATS], F16, kind="ExternalInput")
    wvT = nc.dram_tensor("wvT", [128, 8 * FEATS], F16, kind="ExternalInput")
    woT = nc.dram_tensor("woT", [128, 2 * C], F16, kind="ExternalInput")
    bo = nc.dram_tensor("bo", [128, 8], F32, kind="ExternalInput")

    partial = [nc.dram_tensor(f"partial{c}", [C, CCH], F16) for c in range(NCH)]
    outr = [nc.dram_tensor(f"outr{c}", [FEATS, CCH], F16) for c in range(NCH)]
    outc = [
        nc.dram_tensor(f"outc{c}", [FEATS, CCH], F16, kind="ExternalOutput")
        for c in range(NCH)
    ]
    groups = [[0, 1, 2, 3], [4, 5, 6, 7]]

    with TileContext(nc) as tc:
        with (
            tc.tile_pool(name="const", bufs=1) as cpool,
            tc.tile_pool(name="wts", bufs=1) as wpool,
            tc.tile_pool(name="xload", bufs=3) as xload,
            tc.tile_pool(name="qkv", bufs=1) as qkv,
            tc.tile_pool(name="qcur", bufs=2) as qcur,
            tc.tile_pool(name="expw", bufs=2) as expw,
            tc.tile_pool(name="norm", bufs=2) as norm,
            tc.tile_pool(name="ysbp", bufs=2) as ysbp,
            tc.tile_pool(name="osb", bufs=3) as osb,
            tc.tile_pool(name="pp", bufs=2, space="PSUM") as pp,
            tc.tile_pool(name="sp", bufs=2, space="PSUM") as sp,
            tc.tile_pool(name="yp", bufs=2, space="PSUM") as yp,
        ):
            # ---- startup loads, emitted in first-consumption order: the
            # modeled DMA device is serial, so transfer order ~= emission
            # order across the two HWDGE queues
            wsb = {}
            wsb["q"] = wpool.tile([128, 8 * FEATS], F16, name="w_q")
            xs_pre = []
            xb0t = [
                xload.tile([128, 4 * CCH], F16, name=f"xb0_{h}", tag=f"xb{h}")
                for h in range(2)
            ]
            for half in range(2):
                weng = nc.sync if half == 0 else nc.scalar
                xeng = nc.scalar if half == 0 else nc.sync
                weng.dma_start(
                    out=wsb["q"][:, 1024 * half : 1024 * (half + 1)],
                    in_=wqT[:, 1024 * half : 1024 * (half + 1)],
                )
                xeng.dma_start(
                    out=xb0t[half][:].rearrange("p (k f) -> p k f", f=CCH),
                    in_=xT.ap()[512 * half : 512 * (half + 1), 0:CCH]
                    .rearrange("(k p) f -> p k f", p=128),
                )
                for k in range(4):
                    xs_pre.append(xb0t[half][:, CCH * k : CCH * (k + 1)])
            for sec, wdram in (("k", wkT), ("v", wvT)):
                wt = wpool.tile([128, 8 * FEATS], F16, name=f"w_{sec}")
                for half in range(2):
                    eng = nc.sync if half == 0 else nc.scalar
                    eng.dma_start(
                        out=wt[:, 1024 * half : 1024 * (half + 1)],
                        in_=wdram[:, 1024 * half : 1024 * (half + 1)],
                    )
                wsb[sec] = wt
            wo_sb = wpool.tile([128, 2 * C], F16, name="w_o")
            for m in range(2):
                eng = nc.sync if m == 0 else nc.scalar
                eng.dma_start(
                    out=wo_sb[:, C * m : C * (m + 1)],
                    in_=woT[:, C * m : C * (m + 1)],
                )

            # ---- constants (needed from the v'-transpose / first exp on) ----
            identity = cpool.tile([128, 128], F16)
            make_identity(nc, identity[:])
            # PE warm-up: the p-state ramp needs ~3us of continuous PE busy
            # before full clock; burn it on dummy transposes during the
            # startup DMA wait so the first real matmuls run at 2.4GHz
            warm_in = cpool.tile([128, 128], F16)
            nc.vector.memset(warm_in[:], 0.0)
            warm_ps = pp.tile([128, 128], F16, name="warmps", tag="pp")
            for _ in range(32):
                nc.tensor.matmul(
                    warm_ps[:], warm_in[:], identity[:],
                    is_transpose=True, skip_group_check=True,
                )
            mask = cpool.tile([128, 128], F16)
            make_upper_triangular(nc, mask[:], val=1.0, diag=True)
            bias_sb = cpool.tile([128, 8], F32)
            nc.sync.dma_start(out=bias_sb[:], in_=bo[:, :])

            # ---- persistent activations ----
            kT = [qkv.tile([128, T], F16, name=f"kT_{m}") for m in range(2)]
            # v' tiles, one per key-tile j: [128 keys, 4 heads x (64 v | 1)];
            # the ones col folds the softmax denominator into the AV matmul
            vpj = {}
            vp = {}
            for j in range(KTILES):
                t = qkv.tile([128, 4 * (DH + 1)], F16, name=f"vp_{j}")
                for h in range(NH_CORE):
                    nc.vector.memset(
                        t[:, (DH + 1) * h + DH : (DH + 1) * (h + 1)], 1.0
                    )
                    vp[h, j] = t[:, (DH + 1) * h : (DH + 1) * (h + 1)]
                vpj[j] = t

            def prefetch_x(n):
                t0 = CCH * n
                xss = []
                for half in range(2):
                    xb = xload.tile(
                        [128, 4 * CCH], F16, name=f"xb{n}_{half}", tag=f"xb{half}"
                    )
                    nc.scalar.dma_start(
                        out=xb[:].rearrange("p (k f) -> p k f", f=CCH),
                        in_=xT.ap()[512 * half : 512 * (half + 1), t0 : t0 + CCH]
                        .rearrange("(k p) f -> p k f", p=128),
                    )
                    for k in range(4):
                        xss.append(xb[:, CCH * k : CCH * (k + 1)])
                return xss

            def make_proj_fillers(n, xss):
                """qT tiles + 8 filler closures (one PSUM group each) that
                project chunk n.  Fillers are interleaved between attention
                pairs so the PE stream never drains during exp-bound spans."""
                t0 = CCH * n
                qtiles = [
                    qkv.tile([128, CCH], F16, name=f"qT{n}_{m}") for m in range(2)
                ]
                fillers = []

                def qk_group(sec, m):
                    # split into two half-contraction closures so the filler
                    # spread can place ~0.85us quanta between pairs
                    box = {}

                    def f1():
                        box["ps"] = pp.tile([128, CCH], F32, name="projps", tag="pp")
                        for k in range(4):
                            nc.tensor.matmul(
                                box["ps"][:],
                                wsb[sec][:, 256 * k + 128 * m : 256 * k + 128 * (m + 1)],
                                xss[k][:],
                                start=(k == 0),
                                stop=False,
                            )

                    def f2():
                        ps = box["ps"]
                        for k in range(4, 8):
                            nc.tensor.matmul(
                                ps[:],
                                wsb[sec][:, 256 * k + 128 * m : 256 * k + 128 * (m + 1)],
                                xss[k][:],
                                start=False,
                                stop=(k == 7),
                            )
                        if sec == "q":
                            nc.scalar.copy(out=qtiles[m][:], in_=ps[:])
                        else:
                            nc.vector.tensor_copy(
                                out=kT[m][:, t0 : t0 + CCH], in_=ps[:]
                            )
                    return [f1, f2]

                def v_group(tb):
                    # v directly in key-major layout: stationary = xT k-tile
                    # (tokens as PE columns), moving = wv -> out[token, feat];
                    # one strided DVE copy scatters the 4 heads into v'.
                    box = {}

                    def f1():
                        box["ps"] = pp.tile([128, 4 * DH], F32, name="vtps", tag="pp")
                        for k in range(4):
                            nc.tensor.matmul(
                                box["ps"][:],
                                xss[k][:, 128 * tb : 128 * (tb + 1)],
                                wsb["v"][:, 256 * k : 256 * (k + 1)],
                                start=(k == 0),
                                stop=False,
                            )

                    def f2():
                        j = 4 * n + tb
                        ps = box["ps"]
                        for k in range(4, 8):
                            nc.tensor.matmul(
                                ps[:],
                                xss[k][:, 128 * tb : 128 * (tb + 1)],
                                wsb["v"][:, 256 * k : 256 * (k + 1)],
                                start=False,
                                stop=(k == 7),
                            )
                        nc.vector.tensor_copy(
                            out=vpj[j][:]
                            .rearrange("p (h e) -> p h e", e=DH + 1)[:, :, 0:DH],
                            in_=ps[:].rearrange("p (h d) -> p h d", d=DH),
                        )
                    return [f1, f2]

                for sec in ("q", "k"):
                    for m in range(2):
                        fillers.extend(qk_group(sec, m))
                for tb in range(4):
                    fillers.extend(v_group(tb))
                return qtiles, fillers

            # Attention emitters.  Stage A (scores+exp+mask) streams per
            # (pair, head) into double-buffered per-(h,p) ex tiles.  Stage B
            # runs the AV matmuls TRANSPOSED (stationary = 128-token slice
            # of ex, moving = [v|1]): the output lands token-major
            # [128 tok, 4*(64+1)] so the modeled PE cost is 65 cols per
            # key-tile instead of up-to-512, causal (j,tb) blocks above the
            # diagonal are skipped entirely, and the softmax denominator is
            # a per-PARTITION scalar (cheap reciprocal + tensor_scalar mul,
            # no broadcast matmul).
            def make_attn_state(c, qT):
                return {
                    "c": c,
                    "qT": qT,
                    "exinfo": {},
                    "yT": {},
                    "ysb": [
                        ysbp.tile([128, CCH], F16, name=f"ysb{c}_{m}", tag=f"ysb{m}")
                        for m in range(2)
                    ],
                }

            def emit_pair(st, p, fq=None, slots_left=1):
                c, qT, exinfo = st["c"], st["qT"], st["exinfo"]
                for h in range(NH_CORE):
                    m, b_ = h // 2, h % 2
                    hq = qT[m][64 * b_ : 64 * (b_ + 1), :]
                    hk = kT[m][64 * b_ : 64 * (b_ + 1), :]
                    sc = sp.tile([128, 1024], F32, name="sc", tag="sc")
                    ex = expw.tile(
                        [128, 1024], F16, name=f"ex{h}_{p}", tag=f"ex{h}_{p}"
                    )
                    info = []
                    off = 0
                    for half in range(2):
                        j = 2 * p + half
                        tstart = max(128 * j, CCH * c)
                        w = CCH * (c + 1) - tstart
                        nc.tensor.matmul(
                            sc[0:128, off : off + w],
                            hk[:, 128 * j : 128 * (j + 1)],
                            hq[:, tstart - CCH * c : tstart - CCH * c + w],
                            start=True,
                            stop=True,
                        )
                        info.append((j, tstart, w, off))
                        off += w
                    nc.scalar.activation(
                        ex[:, 0:off],
                        sc[0:128, 0:off],
                        mybir.ActivationFunctionType.Exp,
                        scale=SCALE,
                    )
                    for j, tstart, w, o in info:
                        if 128 * j >= CCH * c:
                            nc.vector.tensor_mul(
                                out=ex[:, o : o + 128],
                                in0=ex[:, o : o + 128],
                                in1=mask[:],
                            )
                        # col of chunk-relative token t is col0 + t
                        exinfo[h, j] = (ex, o - (tstart - CCH * c))

            def stage_b_acc(st, tb, j_lo, j_hi):
                # accumulate key-tiles [j_lo, j_hi] into yT[tb]
                c, exinfo = st["c"], st["exinfo"]
                jmax = 4 * c + tb
                if tb not in st["yT"]:
                    st["yT"][tb] = yp.tile(
                        [128, 4 * (DH + 1)], F32, name=f"yT{tb}", tag="ytp"
                    )
                yT = st["yT"][tb]
                for h in range(NH_CORE):
                    for j in range(j_lo, j_hi + 1):
                        ex, col0 = exinfo[h, j]
                        nc.tensor.matmul(
                            yT[:, (DH + 1) * h : (DH + 1) * (h + 1)],
                            ex[:, col0 + 128 * tb : col0 + 128 * (tb + 1)],
                            vpj[j][:, (DH + 1) * h : (DH + 1) * (h + 1)],
                            start=(j == 0),
                            stop=(j == jmax),
                        )

            def stage_b_fin(st, tb):
                yT = st["yT"].pop(tb)
                ysb = st["ysb"]
                rec = norm.tile([128, 4], F32, name="rec", tag="rec")
                nc.vector.reciprocal(
                    rec[:],
                    yT[:].rearrange("p (h e) -> p h e", e=DH + 1)[:, :, DH : DH + 1],
                )
                yn = norm.tile([128, 4 * DH], F16, name="yn", tag="yn")
                for h in range(NH_CORE):
                    nc.vector.tensor_scalar_mul(
                        out=yn[:, DH * h : DH * (h + 1)],
                        in0=yT[:, (DH + 1) * h : (DH + 1) * h + DH],
                        scalar1=rec[:, h : h + 1],
                    )
                # transpose back to feature-major ysb via PE (rides the
                # pp pool rotation) + DVE copy
                for m in range(2):
                    tp = pp.tile([128, 128], F16, name="ytr", tag="pp")
                    nc.tensor.matmul(
                        tp[:],
                        yn[:, 128 * m : 128 * (m + 1)],
                        identity[:],
                        is_transpose=True,
                    )
                    nc.vector.tensor_copy(
                        out=ysb[m][:, 128 * tb : 128 * (tb + 1)], in_=tp[:]
                    )

            def stage_b(st, tb):
                stage_b_acc(st, tb, 0, 4 * st["c"] + tb)
                stage_b_fin(st, tb)

            def attend_run(st, fillers, start_pair=0, next_st=None, next_pairs=2,
                           tail=False):
                """Emit chunk st's pairs [start_pair..], interleaving fillers,
                with stage-B tb 0,1 at the causal threshold.  The NEXT
                chunk's first two pairs are emitted at the end so ACT has
                exps to run across the chunk boundary; tb 2,3 are deferred
                into the next chunk's filler stream.  In tail mode, tb 2,3
                are pre-accumulated over key-tiles whose exps are already
                COMPLETE at emission (pairs <= 2c-1), so only the last two
                pairs' key-tiles remain after the final exp."""
                c = st["c"]
                npairs = 2 * c + 2
                fq = list(fillers)
                for p in range(start_pair, npairs):
                    emit_pair(st, p)
                    # spread fillers evenly over the remaining pairs
                    nfill = len(fq) // (npairs - p) + (1 if len(fq) % (npairs - p) else 0)
                    for _ in range(min(nfill, len(fq))):
                        fq.pop(0)()
                    if p == max(2 * c, start_pair):
                        stage_b(st, 0)
                        stage_b(st, 1)
                if next_st is not None:
                    for np_ in range(next_pairs):
                        emit_pair(next_st, np_)
                return [lambda: stage_b(st, 2), lambda: stage_b(st, 3)]

            def make_outproj_fillers(c, ysb, act_epilogue=False):
                """4 filler closures, one flush each: 2 o-groups of the
                partial out-projection (own 256 y-feats -> all 1024 outs),
                bias epilogue on DVE, then the partial DMA on the sync queue
                (the scalar queue is reserved for wait-free prefetches so exp
                dispatch on the shared Activation SEQ never stalls)."""
                def flush(t):
                    def f():
                        ob = osb.tile(
                            [128, 2 * CCH], F16, name=f"ob{c}_{t}", tag="ob",
                            bufs=4,
                        )
                        for s in range(2):
                            o = 2 * t + s
                            ps = pp.tile([128, CCH], F32, name="ops", tag="pp")
                            for m in range(2):
                                nc.tensor.matmul(
                                    ps[:],
                                    wo_sb[:, C * m + 128 * o : C * m + 128 * (o + 1)],
                                    ysb[m][:],
                                    start=(m == 0),
                                    stop=(m == 1),
                                )
                            if act_epilogue and s == 1:
                                # final chunk: ACT is idle post-exp, run half
                                # the epilogue there in parallel with DVE
                                nc.scalar.activation(
                                    ob[:, CCH * s : CCH * (s + 1)],
                                    ps[:],
                                    mybir.ActivationFunctionType.Identity,
                                    bias=bias_sb[:, o : o + 1],
                                )
                            else:
                                nc.vector.tensor_scalar_add(
                                    out=ob[:, CCH * s : CCH * (s + 1)],
                                    in0=ps[:],
                                    scalar1=bias_sb[:, o : o + 1],
                                )
                            if act_epilogue:
                                # last chunk: DMA each o-half right after its
                                # own epilogue so the final transfer is half
                                # the size and starts earlier
                                nc.sync.dma_start(
                                    out=partial[c]
                                    .ap()[256 * t + 128 * s : 256 * t + 128 * (s + 1), :]
                                    .rearrange("(k p) f -> p k f", p=128),
                                    in_=ob[:, CCH * s : CCH * (s + 1)]
                                    .rearrange("p (k f) -> p k f", f=CCH),
                                )
                        if not act_epilogue:
                            nc.sync.dma_start(
                                out=partial[c]
                                .ap()[256 * t : 256 * (t + 1), :]
                                .rearrange("(k p) f -> p k f", p=128),
                                in_=ob[:].rearrange("p (k f) -> p k f", f=CCH),
                            )
                    return f

                return [flush(t) for t in range(4)]

            def make_outproj_half(c, ysb, half):
                """Half-token out-projection flushes (tokens 256*half..):
                the first half only needs stage-B tb 0,1 so it runs mid-
                attention; the second half is the post-exp tail, with its
                epilogue split across DVE and the then-idle ACT engine."""
                def flush(t):
                    def f():
                        ob = osb.tile(
                            [128, CCH], F16, name=f"obh{c}_{t}_{half}",
                            tag=f"obh{half}", bufs=2,
                        )
                        for s in range(2):
                            o = 2 * t + s
                            # post-exp flushes borrow the score pool's PSUM
                            # slots (scores are finished) for 2x pipelining
                            pool, ptag = (sp, "sc") if half == 1 and s == 1 else (pp, "pp")
                            ps = pool.tile([128, 256], F32, name="opsh", tag=ptag)
                            for m in range(2):
                                nc.tensor.matmul(
                                    ps[:],
                                    wo_sb[:, C * m + 128 * o : C * m + 128 * (o + 1)],
                                    ysb[m][:, 256 * half : 256 * (half + 1)],
                                    start=(m == 0),
                                    stop=(m == 1),
                                )
                            if half == 1 and s == 1:
                                nc.scalar.activation(
                                    ob[:, 256 * s : 256 * (s + 1)],
                                    ps[:],
                                    mybir.ActivationFunctionType.Identity,
                                    bias=bias_sb[:, o : o + 1],
                                )
                            else:
                                nc.vector.tensor_scalar_add(
                                    out=ob[:, 256 * s : 256 * (s + 1)],
                                    in0=ps[:],
                                    scalar1=bias_sb[:, o : o + 1],
                                )
                        nc.sync.dma_start(
                            out=partial[c]
                            .ap()[256 * t : 256 * (t + 1), 256 * half : 256 * (half + 1)]
                            .rearrange("(k p) f -> p k f", p=128),
                            in_=ob[:].rearrange("p (k f) -> p k f", f=256),
                        )
                    return f

                return [flush(t) for t in range(4)]

            def reduce_scatter(c, last=False):
                # walrus forbids collectives writing IO tensors -> bounce via
                # outr, then copy (both HWDGE queues for the last chunk)
                nc.gpsimd.collective_compute(
                    "ReduceScatter",
                    mybir.AluOpType.add,
                    replica_groups=groups,
                    ins=[partial[c][:].opt()],
                    outs=[outr[c][:].opt()],
                )
                if not last:
                    # Pool queue: sits right behind the RS it waits on, so it
                    # never blocks prefetch/partial dispatch on the HWDGE SEQs
                    nc.gpsimd.dma_start(out=outc[c][:, :], in_=outr[c][:, :])
                else:
                    nc.sync.dma_start(out=outc[c][:, :], in_=outr[c][:, :])

            # Chunks are processed 1,2,3,0: the ACT-bound big chunks sit
            # mid-stream (surrounded by projection/out-projection fillers),
            # and the cheapest chunk (0) forms the tail feeding the final
            # ReduceScatter.  Each attend interleaves as PE fillers: the
            # previous chunk's deferred B-blocks + out-projection flushes +
            # its RS, and the next chunk's projection groups.
            xss = {0: xs_pre}
            qtiles = {}
            qtiles[0], pf0 = make_proj_fillers(0, xss[0])
            for f in pf0:
                f()
            sts = {0: make_attn_state(0, qtiles[0])}
            pending = []
            for c in range(NCH):
                nxt = c + 1 if c + 1 < NCH else None
                if nxt is not None:
                    xss[nxt] = prefetch_x(nxt)
                    qtiles[nxt], pf = make_proj_fillers(nxt, xss[nxt])
                    pending += pf
                    sts[nxt] = make_attn_state(nxt, qtiles[nxt])
                lookahead = {0: 3, 1: 4, 2: 4}  # pairs of chunk c+1 emitted in chunk c
                bdef = attend_run(
                    sts[c],
                    pending,
                    start_pair=0 if c == 0 else lookahead[c - 1],
                    next_st=sts.get(nxt) if nxt is not None else None,
                    next_pairs=lookahead.get(c, 0),
                    tail=(nxt is None),
                )
                pending = bdef + make_outproj_fillers(
                    c, sts[c]["ysb"], act_epilogue=(nxt is None)
                )
                pending.append(
                    lambda c=c, last=(nxt is None): reduce_scatter(c, last)
                )
            for f in pending:
                f()

    return nc


_PROGRAM = None


def _get_program():
    global _PROGRAM
    if _PROGRAM is None:
        _apply_walrus_workaround()
        _PROGRAM = _build_program()
    return _PROGRAM


def kernel(x, w_qkv, w_out, b_out):
    x = np.asarray(x, dtype=np.float32)
    w_qkv = np.asarray(w_qkv, dtype=np.float32)
    w_out = np.asarray(w_out, dtype=np.float32)
    b_out = np.asarray(b_out, dtype=np.float32)

    bias_tile = np.ascontiguousarray(b_out.reshape(8, 128).T)  # [128, 8]
    zeros_tile = np.zeros_like(bias_tile)

    def sb_layout(wT):  # [1024or256, F] -> [128, (k f)] SBUF layout
        k = wT.shape[0] // 128
        return np.ascontiguousarray(
            wT.reshape(k, 128, -1).transpose(1, 0, 2).reshape(128, -1)
        )

    in_maps = []
    for i in range(N_CORES):
        b, g = divmod(i, 4)
        sl = slice(FEATS * g, FEATS * (g + 1))
        in_maps.append(
            {
                "xT": np.ascontiguousarray(x[b].T.astype(np.float16)),
                "wqT": sb_layout(w_qkv[0 * C :][sl].T.astype(np.float16)),
                "wkT": sb_layout(w_qkv[1 * C :][sl].T.astype(np.float16)),
                "wvT": sb_layout(w_qkv[2 * C :][sl].T.astype(np.float16)),
                "woT": sb_layout(w_out[:, sl].T.astype(np.float16)),
                "bo": bias_tile if g == 0 else zeros_tile,
            }
        )

    nc = _get_program()
    res = run_bass_kernel_spmd(nc, in_maps, core_ids=list(range(N_CORES)))
    kernel.last_results = res

    outs = []
    for b in range(B):
        full = np.empty((C, T), dtype=np.float32)
        for g in range(4):
            r = res.results[4 * b + g]
            for c in range(NCH):
                full[FEATS * g : FEATS * (g + 1), CCH * c : CCH * (c + 1)] = r[
                    f"outc{c}"
                ].astype(np.float32)
        outs.append(full.T)
    return np.stack(outs)

